# revision 16
# baseline (speedup 1.0000x reference)
"""Trainium2 Bass kernel for nn_MIGAModel (moe_routing).

Pure data parallel over the stock axis N (8 cores, 2500 rows each).

Router precision scheme (the top-2 gating is discontinuous in the router
logits h, so h must be fp32-accurate to ~1e-5; plain fp16/bf16/fp32r
inputs all flip expert selections and fail the 2e-2 gate):
    x  = a + b      a = fp16(x),  b8 = fp8e4m3(b * 2048)
    Wr = c + d      c = fp16(Wr), d16 = fp16(d * 2048)
    h  = a@c  +  (a@d16 + b8@c8) / 2048        (c8 = fp8(c))
Two fp16 passes at 1 cyc/row plus one fp8 DoubleRow pass at 0.5 cyc/row
(pairs of K-chunks per instruction), two PSUM banks (main, aux), one
ACT + one DVE op to combine.  The router bias br rides in as an extra
contraction row (a row of ones in `a`, br split across c/d16), so
selection sees the exact biased logits.  delta-h ~1e-5 -> end-to-end
rel err ~3e-3 (selection flips dominate; measured in numpy and on HW).

DMA: a is 2 B/elem, b8 1 B/elem -> ~72 MB/core vs 95 MB for fp32.
Post-processing (experts + inner-group attention as 128x128
block-diagonal matmuls) runs on bf16 operands (1 cyc/row), biases are
folded into ACT-engine PSUM->SBUF moves.  The top-2 threshold test runs
in the PE-transposed space (rows on partitions) where the per-row
second-max is a [128,1] tensor_scalar operand, and the 0/1 mask is
transposed back by cheap bf16 PE transposes — no fp32 broadcast
matmuls.

Scheduling: the post chain of chunk c is a latency-bound
PE<->DVE<->ACT ping-pong, so its PE instructions are interleaved into
chunk c+1's router matmul stream in small groups — the tensor engine
never idles (which would also reset the cost model's p-state ramp).
Chunk widths taper at both ends ([344,500,500,500,400,256]) so the
first matmul starts early in the DMA stream and
final, un-hidden post chain is as short as possible.  Output DMAs
issue from the ACT queue so the SP queue (x tiles) keeps streaming.
"""
import sys
import numpy as np

for _p in ("/opt/trn_rl_repo",):
    if _p not in sys.path:
        sys.path.insert(0, _p)

import ml_dtypes

import concourse.bass as bass
import concourse.tile as tile
from concourse import bacc, mybir
from concourse.bass_utils import run_bass_kernel_spmd

F32 = mybir.dt.float32
F16 = mybir.dt.float16
F8 = mybir.dt.float8e4
BF16 = mybir.dt.bfloat16

N, T, D = 20000, 60, 158
TD = T * D                      # 9480
G, E, H, DH, GE = 8, 16, 4, 4, 128
NCORES = 8
NSH = N // NCORES               # 2500 rows per core
KT = 75                         # fp16 K-chunks of 128 (9600 padded, row 9480 = ones)
TDP = KT * 128                  # 9600
KT2 = 38                        # fp8 K-pairs (9728 padded)
NQ = 5                          # a-tile K-groups per chunk (15 K-chunks each)
KQ = KT // NQ                   # 15
RS = 2048.0                     # residual scale (2**11)

# compute chunk widths: tapered so the last (un-hidden) post chain is short;
# every width is >=256 so fp16 a-tile DMA runs stay >=512 B.
WIDTHS = [344, 500, 500, 500, 400, 256]
NCH = len(WIDTHS)
LOS = [sum(WIDTHS[:i]) for i in range(NCH)]
# fp8 per-sub-row padded widths (pair stride must be a multiple of 16 B)
W8S = [(w + 15) // 16 * 16 for w in WIDTHS]
B8OFF = [2 * sum(W8S[:i]) for i in range(NCH)]
B8TOT = 2 * sum(W8S)

USE_DOUBLE_ROW = True

# bf16 packed matrix indices ([128,128] blocks in "mats16")
M_WET, M_AQ = 0, 1
M_AK0, M_AV0 = 2, 6             # 4 each
M_MS0 = 10                      # 4
M_MDEN = 14
M_MER0 = 15                     # 4
M_AO = 19
M_IDT = 20
M_ONES = 21
NM16 = 22

# fp32 packed matrices: identity (fp32 transposes)
M32_IDT = 0
NM32 = 1

# bias pack columns (fp32)
B_BE, B_BQ, B_BK0, B_BV0, B_BO = 0, 1, 2, 6, 10
NBIAS = 11


def build_consts(Wr, br, We, be, Wq, bq, Wk, bk, Wv, bv, Wo, bo):
    """Host-side packed constants (see build_kernel for layouts)."""
    f32 = np.float32
    Wr = np.asarray(Wr, f32)
    br = np.asarray(br, f32)
    We = np.asarray(We, f32)
    be = np.asarray(be, f32)
    Wq = np.asarray(Wq, f32)
    bq = np.asarray(bq, f32)
    Wk = np.asarray(Wk, f32)
    bk = np.asarray(bk, f32)
    Wv = np.asarray(Wv, f32)
    bv = np.asarray(bv, f32)
    Wo = np.asarray(Wo, f32)
    bo = np.asarray(bo, f32)

    # router weight split; bias br rides on the ones-row (index TD)
    w_full = np.zeros((KT2 * 256, GE), f32)
    w_full[:TD] = Wr
    w_full[TD] = br
    c_full = w_full.astype(np.float16).astype(f32)
    d_full = ((w_full - c_full) * RS).astype(np.float16).astype(f32)

    def pmajor(a, kt):  # [kt*128, GE] -> [128, kt*128] partition-major
        return np.ascontiguousarray(
            a[:kt * 128].reshape(kt, 128, GE).transpose(1, 0, 2).reshape(128, kt * GE))

    c16 = pmajor(c_full, KT).astype(np.float16)
    d16s = pmajor(d_full, KT).astype(np.float16)
    c8 = pmajor(c_full, KT2 * 2).astype(ml_dtypes.float8_e4m3fn)

    mats = np.zeros((NM16, GE, GE), f32)
    biasp = np.zeros((GE, NBIAS), f32)

    mats[M_WET] = np.transpose(We, (2, 0, 1)).reshape(GE, GE)
    biasp[:, B_BE] = be.reshape(GE)
    biasp[:, B_BO] = bo.reshape(GE)

    d_ = np.arange(DH)
    for g in range(G):
        for h in range(H):
            for d in range(DH):
                p = d * 32 + g * 4 + h
                mats[M_AQ, g * 16:(g + 1) * 16, p] = Wq[g, h * 4 + d, :]
                biasp[p, B_BQ] = bq[g, h * 4 + d]
            for e in range(DH):
                ps = d_ * 32 + g * 4 + h
                for p in ps:
                    mats[M_AK0 + e, g * 16:(g + 1) * 16, p] = Wk[g, h * 4 + e, :]
                    mats[M_AV0 + e, g * 16:(g + 1) * 16, p] = Wv[g, h * 4 + e, :]
                    biasp[p, B_BK0 + e] = bk[g, h * 4 + e]
                    biasp[p, B_BV0 + e] = bv[g, h * 4 + e]
    for e in range(DH):
        for d in range(DH):
            for g in range(G):
                for h in range(H):
                    mats[M_MS0 + e, d * 32 + g * 4 + h, e * 32 + d * 8 + g] = 1.0
                    mats[M_MDEN, e * 32 + d * 8 + g, d * 32 + g * 4 + h] = 1.0
                    mats[M_MER0 + e, e * 32 + d * 8 + g, d * 32 + g * 4 + h] = 1.0
    for g in range(G):
        for f in range(E):
            for h in range(H):
                for d in range(DH):
                    mats[M_AO, d * 32 + g * 4 + h, g * 16 + f] = Wo[g, f, h * 4 + d]
    mats[M_IDT] = np.eye(GE, dtype=f32)
    mats[M_ONES] = 1.0

    mats16 = np.ascontiguousarray(
        np.transpose(mats, (1, 0, 2)).reshape(GE, NM16 * GE)).astype(ml_dtypes.bfloat16)

    m32 = np.zeros((NM32, GE, GE), f32)
    m32[M32_IDT] = np.eye(GE, dtype=f32)
    mats32 = np.ascontiguousarray(np.transpose(m32, (1, 0, 2)).reshape(GE, NM32 * GE))
    return c16, d16s, c8, mats16, mats32, biasp


def prep_x_shard(xs):
    """xs [NSH, TD] fp32 -> (a16 [TDP, NSH] fp16, b8 [KT2, 128, B8TOT] fp8).

    a16 row TD is all-ones (carries the router bias); b8 is the scaled
    residual (x - fp16(x)) * 2048, pair-of-K-chunks packed and column
    pre-blocked per compute chunk (chunk widths padded per sub-row so
    the DoubleRow pair stride is a multiple of 16 B and DMA runs are
    >=512 B).
    """
    f32 = np.float32
    xt = np.zeros((KT2 * 256, NSH), f32)
    xt[:TD] = xs.T
    xt[TD] = 1.0                            # ones-row carries the router bias
    a = xt[:TDP].astype(np.float16)         # row TD: fp16(1.0) exact
    b = xt * RS
    b[:TDP] = (xt[:TDP] - a.astype(f32)) * RS   # rows TD.. stay 0
    b8s = np.asarray(b.astype(ml_dtypes.float8_e4m3fn))  # [KT2*256, NSH]
    b8s = b8s.reshape(KT2, 2, 128, NSH)
    b8 = np.zeros((KT2, 128, B8TOT), ml_dtypes.float8_e4m3fn)
    for c in range(NCH):
        lo, w, w8, off = LOS[c], WIDTHS[c], W8S[c], B8OFF[c]
        for s in range(2):
            b8[:, :, off + s * w8: off + s * w8 + w] = b8s[:, s, :, lo:lo + w]
    return np.ascontiguousarray(a), b8


def build_kernel():
    """Trace the Bass/Tile kernel; returns the compiled Bacc."""
    nc = bacc.Bacc("TRN2", target_bir_lowering=False, debug=False,
                   num_devices=NCORES)

    a_d = nc.dram_tensor("a16", [TDP, NSH], F16, kind="ExternalInput").ap()
    b_d = nc.dram_tensor("b8", [KT2, 128, B8TOT], F8, kind="ExternalInput").ap()
    c16_d = nc.dram_tensor("c16", [128, KT * 128], F16, kind="ExternalInput").ap()
    d16_d = nc.dram_tensor("d16s", [128, KT * 128], F16, kind="ExternalInput").ap()
    c8_d = nc.dram_tensor("c8", [128, KT2 * 256], F8, kind="ExternalInput").ap()
    m16_d = nc.dram_tensor("mats16", [128, NM16 * 128], BF16, kind="ExternalInput").ap()
    m32_d = nc.dram_tensor("mats32", [128, NM32 * 128], F32, kind="ExternalInput").ap()
    bias_d = nc.dram_tensor("bias", [128, NBIAS], F32, kind="ExternalInput").ap()
    out_d = nc.dram_tensor("out", [1, NSH], F32, kind="ExternalOutput").ap()

    AFT = mybir.ActivationFunctionType

    with tile.TileContext(nc) as tc:
        with (
            tc.tile_pool(name="consts", bufs=1) as consts,
            tc.tile_pool(name="xa", bufs=4) as xa,
            tc.tile_pool(name="xb", bufs=2) as xb,
            tc.tile_pool(name="work", bufs=1) as work,
            tc.tile_pool(name="rt", bufs=4, space="PSUM") as rtp,
            tc.tile_pool(name="pt", bufs=4, space="PSUM") as ptp,
        ):
            # ---- constant tiles (DMAs issued inside the chunk-0 stream) ----
            c16_sb = consts.tile([128, KT, 128], F16, tag="c16")
            d16_sb = consts.tile([128, KT, 128], F16, tag="d16")
            c8_sb = consts.tile([128, KT2 * 2, 128], F8, tag="c8")
            m16_sb = consts.tile([128, NM16 * 128], BF16, tag="m16")
            m32_sb = consts.tile([128, NM32 * 128], F32, tag="m32")
            bias_sb = consts.tile([128, NBIAS], F32, tag="bias")

            def mat16(i):
                return m16_sb[:, i * 128:(i + 1) * 128]

            def bcol(i):
                return bias_sb[:, i:i + 1]

            idt32 = m32_sb[:, M32_IDT * 128:(M32_IDT + 1) * 128]
            idt16 = mat16(M_IDT)
            ones16 = mat16(M_ONES)

            def dma_cd_piece(k):
                """k-th quarter of the c16/d16s constants (19 K-chunks)."""
                t0, t1 = k * 19, min(KT, (k + 1) * 19)
                nc.sync.dma_start(
                    out=c16_sb[:, t0:t1, :],
                    in_=c16_d[:, t0 * 128:t1 * 128].rearrange(
                        "p (t m) -> p t m", m=128))
                nc.sync.dma_start(
                    out=d16_sb[:, t0:t1, :],
                    in_=d16_d[:, t0 * 128:t1 * 128].rearrange(
                        "p (t m) -> p t m", m=128))

            def router_thunks(c):
                """DMA + matmul thunk list for chunk c's router passes."""
                lo, w, w8, boff = LOS[c], WIDTHS[c], W8S[c], B8OFF[c]
                sl = slice(lo, lo + w)
                main_ps = rtp.tile([128, w], F32, tag="rt", name=f"main{c}")
                aux_ps = rtp.tile([128, w], F32, tag="rt", name=f"aux{c}")
                thunks = []
                atiles = [None] * NQ
                btiles = [None, None]

                def dma_a(q):
                    at = xa.tile([128, KQ, w], F16, tag="a", name=f"a{c}_{q}")
                    nc.sync.dma_start(
                        out=at,
                        in_=a_d[q * KQ * 128:(q + 1) * KQ * 128, sl].rearrange(
                            "(t p) j -> p t j", p=128))
                    atiles[q] = at

                def dma_b(hh):
                    t2n = KT2 // 2
                    bt = xb.tile([128, t2n, 2 * w8], F8, tag="b", name=f"b{c}_{hh}")
                    nc.sync.dma_start(
                        out=bt,
                        in_=b_d[hh * t2n:(hh + 1) * t2n, :,
                                boff:boff + 2 * w8].rearrange("t p m -> p t m"))
                    btiles[hh] = bt

                for q in range(NQ):
                    def pre(q=q):
                        dma_a(q)
                        if c == 0:
                            if q < 4:
                                dma_cd_piece(q)
                            if q == 3:
                                nc.sync.dma_start(
                                    out=c8_sb,
                                    in_=c8_d.rearrange("p (t m) -> p t m", m=128))
                                dma_b(0)
                            elif q == 4:
                                dma_b(1)
                                nc.sync.dma_start(out=m16_sb, in_=m16_d)
                                nc.sync.dma_start(out=m32_sb, in_=m32_d)
                                nc.sync.dma_start(out=bias_sb, in_=bias_d)
                        else:
                            if q == 3:
                                dma_b(0)
                            elif q == 4:
                                dma_b(1)

                    for t in range(KQ):
                        def mm_main(q=q, t=t, pre=(pre if t == 0 else None)):
                            if pre:
                                pre()
                            tg = q * KQ + t
                            nc.tensor.matmul(main_ps[:, :], lhsT=c16_sb[:, tg, :],
                                             rhs=atiles[q][:, t, :],
                                             start=(tg == 0), stop=(tg == KT - 1))
                        thunks.append(mm_main)
                    for t in range(KQ):
                        def mm_aux(q=q, t=t):
                            tg = q * KQ + t
                            nc.tensor.matmul(aux_ps[:, :], lhsT=d16_sb[:, tg, :],
                                             rhs=atiles[q][:, t, :],
                                             start=(tg == 0), stop=False)
                        thunks.append(mm_aux)

                b_thunks = []
                if USE_DOUBLE_ROW:
                    for g in range(KT2):
                        def mm_b(g=g):
                            hh, t2 = divmod(g, KT2 // 2)
                            rhs = btiles[hh][:, t2, :].rearrange(
                                "p (s j) -> p s j", s=2)[:, :, 0:w]
                            nc.tensor.matmul(
                                aux_ps[:, :], lhsT=c8_sb[:, 2 * g:2 * g + 2, :],
                                rhs=rhs, start=False, stop=(g == KT2 - 1),
                                perf_mode=mybir.MatmulPerfMode.DoubleRow)
                        b_thunks.append(mm_b)
                else:
                    for g in range(KT2):
                        for s in range(2):
                            def mm_b(g=g, s=s):
                                hh, t2 = divmod(g, KT2 // 2)
                                nc.tensor.matmul(
                                    aux_ps[:, :], lhsT=c8_sb[:, 2 * g + s, :],
                                    rhs=btiles[hh][:, t2, s * w8:s * w8 + w],
                                    start=False,
                                    stop=(g == KT2 - 1 and s == 1))
                            b_thunks.append(mm_b)
                return thunks, b_thunks, main_ps, aux_ps

            def post_groups(c, main_ps, aux_ps):
                """Post chain for chunk c as (frac, thunk) groups.

                frac positions the group inside chunk c+1's router matmul
                stream; PE members' dependencies are produced well before
                the PE reaches them, so the tensor engine never stalls.
                """
                lo, w = LOS[c], WIDTHS[c]
                blks = [(off, min(128, w - off)) for off in range(0, w, 128)]
                st = {}

                def g_h():
                    aux_sb = work.tile([128, w], F32, tag="auxs", name="auxs")
                    nc.scalar.activation(aux_sb, aux_ps[:, :], AFT.Identity,
                                         scale=1.0 / RS)
                    st["h"] = work.tile([128, w], F32, tag="h", name="h")
                    nc.vector.tensor_add(st["h"], main_ps[:, :], aux_sb)
                    st["h16"] = work.tile([128, w], BF16, tag="h16", name="h16")
                    nc.scalar.activation(st["h16"], st["h"], AFT.Copy)

                def g_top2():
                    # per-row top-2 threshold + mask, in transposed space
                    st["trs"] = []
                    for blk, (off, cs) in enumerate(blks):
                        tr = ptp.tile([128, 128], F32, tag="pt", name=f"tr{c}_{blk}")
                        nc.tensor.transpose(tr[:cs, :GE], st["h"][:, off:off + cs],
                                            idt32)
                        mx1 = work.tile([128, 1], F32, tag="mx1", name="mx1")
                        nc.vector.reduce_max(mx1[:cs], tr[:cs, :GE],
                                             axis=mybir.AxisListType.X)
                        eqm = work.tile([128, GE], F32, tag="eqm", name="eqm")
                        nc.vector.tensor_scalar(eqm[:cs], tr[:cs, :GE], mx1[:cs],
                                                None, op0=mybir.AluOpType.is_ge)
                        hm = work.tile([128, GE], F32, tag="hm", name="hm")
                        nc.vector.scalar_tensor_tensor(
                            hm[:cs], in0=eqm[:cs], scalar=-1e30, in1=tr[:cs, :GE],
                            op0=mybir.AluOpType.mult, op1=mybir.AluOpType.add)
                        mx2 = work.tile([128, 1], F32, tag="mx2", name="mx2")
                        nc.vector.reduce_max(mx2[:cs], hm[:cs],
                                             axis=mybir.AxisListType.X)
                        mtr = work.tile([128, GE], BF16, tag="mtr", name="mtr")
                        nc.vector.tensor_scalar(mtr[:cs], tr[:cs, :GE], mx2[:cs],
                                                None, op0=mybir.AluOpType.is_ge)
                        st["trs"].append((mtr, off, cs))

                def g_maskback():
                    st["mask_ps"] = ptp.tile([128, w], BF16, tag="pt",
                                             name=f"maskps{c}")
                    for mtr, off, cs in st["trs"]:
                        nc.tensor.transpose(st["mask_ps"][:GE, off:off + cs],
                                            mtr[:cs, :GE], idt16[:cs, :cs])

                def g_gate():
                    eh16 = work.tile([128, w], BF16, tag="eh", name="eh")
                    nc.scalar.activation(eh16, st["h"], AFT.Exp)
                    st["m1"] = work.tile([128, w], BF16, tag="m1", name="m1")
                    nc.vector.tensor_mul(st["m1"], eh16, st["mask_ps"][:, :])

                def g_eo():
                    eo_ps = ptp.tile([128, w], F32, tag="pt", name=f"eo{c}")
                    nc.tensor.matmul(eo_ps[:, :], lhsT=mat16(M_WET), rhs=st["h16"],
                                     start=True, stop=True)
                    st["eo16"] = work.tile([128, w], BF16, tag="eo", name="eo")
                    nc.scalar.activation(st["eo16"], eo_ps[:, :], AFT.Identity,
                                         bias=bcol(B_BE), scale=1.0)

                def g_q():
                    q_ps = ptp.tile([128, w], F32, tag="pt", name=f"q{c}")
                    nc.tensor.matmul(q_ps[:, :], lhsT=mat16(M_AQ), rhs=st["eo16"],
                                     start=True, stop=True)
                    st["qt16"] = work.tile([128, w], BF16, tag="qt", name="qt")
                    nc.scalar.activation(st["qt16"], q_ps[:, :], AFT.Identity,
                                         bias=bcol(B_BQ), scale=1.0)
                    st["sc_ps"] = ptp.tile([128, w], F32, tag="pt", name=f"sc{c}")

                def g_kr(e):
                    kr_ps = ptp.tile([128, w], F32, tag="pt", name=f"kr{c}_{e}")
                    nc.tensor.matmul(kr_ps[:, :], lhsT=mat16(M_AK0 + e),
                                     rhs=st["eo16"], start=True, stop=True)
                    kr16 = work.tile([128, w], BF16, tag="kr", name="kr")
                    nc.scalar.activation(kr16, kr_ps[:, :], AFT.Identity,
                                         bias=bcol(B_BK0 + e), scale=1.0)
                    pe16 = work.tile([128, w], BF16, tag=f"pe{e}", name=f"pe{e}")
                    nc.vector.tensor_mul(pe16, st["qt16"], kr16)
                    st[f"pe{e}"] = pe16

                def g_ms(e):
                    nc.tensor.matmul(st["sc_ps"][:, :], lhsT=mat16(M_MS0 + e),
                                     rhs=st[f"pe{e}"], start=(e == 0),
                                     stop=(e == DH - 1))
                    if e == DH - 1:
                        st["es16"] = work.tile([128, w], BF16, tag="es", name="es")
                        nc.scalar.activation(st["es16"], st["sc_ps"][:, :],
                                             AFT.Exp, scale=0.5)

                def g_den():
                    den_ps = ptp.tile([128, w], F32, tag="pt", name=f"den{c}")
                    nc.tensor.matmul(den_ps[:, :], lhsT=mat16(M_MDEN),
                                     rhs=st["es16"], start=True, stop=True)
                    st["drec"] = work.tile([128, w], F32, tag="drec", name="drec")
                    nc.vector.reciprocal(st["drec"], den_ps[:, :])

                def g_vr(e):
                    vr_ps = ptp.tile([128, w], F32, tag="pt", name=f"vr{c}_{e}")
                    nc.tensor.matmul(vr_ps[:, :], lhsT=mat16(M_AV0 + e),
                                     rhs=st["eo16"], start=True, stop=True)
                    vr16 = work.tile([128, w], BF16, tag="vr", name="vr")
                    nc.scalar.activation(vr16, vr_ps[:, :], AFT.Identity,
                                         bias=bcol(B_BV0 + e), scale=1.0)
                    er_ps = ptp.tile([128, w], F32, tag="pt", name=f"er{c}_{e}")
                    nc.tensor.matmul(er_ps[:, :], lhsT=mat16(M_MER0 + e),
                                     rhs=st["es16"], start=True, stop=True)
                    pr16 = work.tile([128, w], BF16, tag=f"pr{e}", name=f"pr{e}")
                    nc.vector.tensor_mul(pr16, er_ps[:, :], vr16)
                    st[f"pr{e}"] = pr16

                def g_att():
                    s1 = work.tile([128, w], BF16, tag="s1", name="s1")
                    nc.vector.tensor_add(s1, st["pr0"], st["pr1"])
                    s2 = work.tile([128, w], BF16, tag="s2", name="s2")
                    nc.vector.tensor_add(s2, st["pr2"], st["pr3"])
                    s3 = work.tile([128, w], BF16, tag="s3", name="s3")
                    nc.vector.tensor_add(s3, s1, s2)
                    st["att16"] = work.tile([128, w], BF16, tag="att", name="att")
                    nc.vector.tensor_mul(st["att16"], s3, st["drec"])

                def g_ao():
                    ao_ps = ptp.tile([128, w], F32, tag="pt", name=f"ao{c}")
                    nc.tensor.matmul(ao_ps[:, :], lhsT=mat16(M_AO),
                                     rhs=st["att16"], start=True, stop=True)
                    aout16 = work.tile([128, w], BF16, tag="aout", name="aout")
                    nc.scalar.activation(aout16, ao_ps[:, :], AFT.Identity,
                                         bias=bcol(B_BO), scale=1.0)
                    st["num16"] = work.tile([128, w], BF16, tag="num", name="num")
                    nc.vector.tensor_mul(st["num16"], st["m1"], aout16)

                def g_fin():
                    dens_ps = ptp.tile([1, w], F32, tag="pt", name=f"dens{c}")
                    nc.tensor.matmul(dens_ps[:, :], lhsT=ones16[:, 0:1],
                                     rhs=st["m1"], start=True, stop=True)
                    nums_ps = ptp.tile([1, w], F32, tag="pt", name=f"nums{c}")
                    nc.tensor.matmul(nums_ps[:, :], lhsT=ones16[:, 0:1],
                                     rhs=st["num16"], start=True, stop=True)
                    rden = work.tile([1, w], F32, tag="rden", name="rden")
                    nc.vector.reciprocal(rden, dens_ps[:, :])
                    pred = work.tile([1, w], F32, tag="pred", name="pred", bufs=2)
                    nc.vector.tensor_mul(pred, nums_ps[:, :], rden)
                    nc.scalar.dma_start(out=out_d[0:1, lo:lo + w], in_=pred)

                return [
                    (0.00, g_h),
                    (0.06, g_top2),
                    (0.17, g_maskback),
                    (0.21, g_gate),
                    (0.25, g_eo),
                    (0.31, g_q),
                    (0.35, lambda: g_kr(0)),
                    (0.40, lambda: (g_ms(0), g_kr(1))),
                    (0.45, lambda: (g_ms(1), g_kr(2))),
                    (0.50, lambda: (g_ms(2), g_kr(3))),
                    (0.55, lambda: g_ms(3)),
                    (0.61, lambda: (g_den(), g_vr(0))),
                    (0.66, lambda: g_vr(1)),
                    (0.71, lambda: g_vr(2)),
                    (0.76, lambda: g_vr(3)),
                    (0.80, g_att),
                    (0.85, g_ao),
                    (0.92, g_fin),
                ]

            # chunk c's fp8 b-pass and post chain both run inside chunk
            # c+1's fp16 matmul stream (the aux accumulation group stays
            # open across the chunk boundary), so PE never waits on the
            # b-tile DMAs or the post chain's DVE/ACT producers.
            pending_b = None
            pending_post = None
            for c in range(NCH):
                thunks, b_thunks, main_ps, aux_ps = router_thunks(c)
                nmm = len(thunks)
                sched = {}
                if pending_b is not None:
                    sched.setdefault(int(0.05 * nmm), []).extend(pending_b)
                if pending_post is not None:
                    for frac, fn in pending_post:
                        pos = min(nmm - 1, int((0.17 + 0.81 * frac) * nmm))
                        sched.setdefault(pos, []).append(fn)
                for i, t in enumerate(thunks):
                    t()
                    for fn in sched.get(i, ()):
                        fn()
                pending_b = b_thunks
                pending_post = post_groups(c, main_ps, aux_ps)
            for fn in pending_b:
                fn()
            for frac, fn in pending_post:
                fn()

    nc.compile()
    return nc


_NC_CACHE = None
LAST_RESULTS = None


def kernel(x, Wr, br, We, be, Wq, bq, Wk, bk, Wv, bv, Wo, bo):
    global _NC_CACHE, LAST_RESULTS
    f32 = np.float32
    x = np.asarray(x, f32)

    c16, d16s, c8, mats16, mats32, biasp = build_consts(
        Wr, br, We, be, Wq, bq, Wk, bk, Wv, bv, Wo, bo)

    if _NC_CACHE is None:
        _NC_CACHE = build_kernel()
    nc = _NC_CACHE

    in_maps = []
    for c in range(NCORES):
        xs = x[c * NSH:(c + 1) * NSH].reshape(NSH, TD)
        a16, b8 = prep_x_shard(xs)
        in_maps.append({"a16": a16, "b8": b8, "c16": c16, "d16s": d16s,
                        "c8": c8, "mats16": mats16, "mats32": mats32,
                        "bias": biasp})

    res = run_bass_kernel_spmd(nc, in_maps, list(range(NCORES)))
    LAST_RESULTS = res
    out = np.concatenate([res.results[c]["out"].reshape(NSH)
                          for c in range(NCORES)])
    return out.astype(f32)
